# revision 16
# baseline (speedup 1.0000x reference)
"""CLIPAttention sparse-attention kernel for 8 Trainium2 NeuronCores.

Problem shapes: B=4, T=4097 (1 global token + 16 frames x 256), C=1024, H=16.

Sharding: core = b*2 + th  (b = batch 0..3, th = token-half 0..1).
Each core handles ALL 16 heads for its 2048 frame tokens (8 frames) plus a
copy of the global token.  The only cross-core coupling is the single global
query (token 0), handled by unnormalized partial sums (oG, zG) that the host
combines (a ~4KB reduce + one 1024x1024 matvec per batch).

Device-side per core:
  stage A: hsT [C, 2049] (bf16, global token at local col 0) -> qT, kT
           (feature-major, kT in a padded per-frame layout with the global
           key replicated at col 256 of each 257-wide frame block), v
           (token-major), plus q0/k0/v0 rows for the global token.
  stage B: per (frame, head): S = qT^T kT_block [128q, 257], exp on ACT with
           accum_out giving the softmax denominator, normalize on DVE,
           PE-transpose the 2x128 local columns, PV matmuls -> O^T (bf16),
           global-key attention weight column collected into C [128, 17].
  stage C: global-query partials via block-diagonal q0 rhs: sGT [128t, 16],
           exp, denominators via ones-matmul, oG via paired-head matmuls.
  stage D: Wg = blockdiag(v0) @ WoT (+bo row), Y = OT^T WoT + C^T Wg per
           128-token tile, DMA out.
"""

import numpy as np
import ml_dtypes
from contextlib import ExitStack

import concourse.bass as bass
import concourse.bacc as bacc
import concourse.tile as tile
import concourse.mybir as mybir
from concourse.bass_utils import run_bass_kernel_spmd
from concourse.masks import make_identity

BF16 = mybir.dt.bfloat16
F32 = mybir.dt.float32
AF = mybir.ActivationFunctionType
NPBF16 = ml_dtypes.bfloat16

B, T, C, H, DH = 4, 4097, 1024, 16, 64
NFRAME, L = 16, 256  # total frames, frame length
SCALE = DH ** -0.5
FT = 2048            # frame tokens per core
TL = FT + 1          # local tokens (global token at local col 0)
NFL = 8              # frames per core
NCORES = 8
LB = L + 1           # padded key block: 256 frame keys + 1 global key


def _emit(ctx: ExitStack, tc, io):
    nc = tc.nc

    # ---------------- constant / small tiles ----------------
    const_pool = ctx.enter_context(tc.tile_pool(name="const", bufs=1))
    ident = const_pool.tile([128, 128], BF16, tag="ident")
    make_identity(nc, ident[:])
    ones1 = const_pool.tile([1, 128], BF16, tag="ones1")
    nc.gpsimd.memset(ones1[:], 1.0)
    onesc = const_pool.tile([128, 1], BF16, tag="onesc")
    nc.gpsimd.memset(onesc[:], 1.0)

    gq = const_pool.tile([1, 1], F32, tag="gq")
    nc.sync.dma_start(gq[:], io["gqm"][:, :])

    # biases: column form [128, 8] f32 (feature-major) and row form bf16
    bcol = {}
    brow = {}
    with tc.tile_pool(name="brf", bufs=2) as brf_pool:
        for nm in ("bq", "bk", "bv", "bo"):
            t = const_pool.tile([128, 8], F32, tag=f"{nm}c")
            nc.sync.dma_start(t[:], io[nm].rearrange("(jc p) -> p jc", p=128))
            bcol[nm] = t
            rf = brf_pool.tile([1, C], F32, tag="brf")
            nc.sync.dma_start(rf[:], io[nm].rearrange("(a j) -> a j", a=1))
            rb = const_pool.tile([1, C], BF16, tag=f"{nm}r")
            nc.scalar.copy(rb[:], rf[:])
            brow[nm] = rb

    # ---------------- persistent big tiles ----------------
    big_pool = ctx.enter_context(tc.tile_pool(name="big", bufs=1))
    qT = big_pool.tile([128, 8, TL], BF16, tag="qT")
    kTp = big_pool.tile([128, 8, NFL * LB], BF16, tag="kTp")
    v = big_pool.tile([128, 16, C], BF16, tag="v")
    v0 = big_pool.tile([1, C], BF16, tag="v0")
    ogp = big_pool.tile([128, 8], F32, tag="ogp")
    zg = big_pool.tile([16, 1], F32, tag="zg")
    woT = big_pool.tile([128, 8, C], BF16, tag="woT")
    wg = big_pool.tile([H + 1, C], BF16, tag="wg")
    vgm = big_pool.tile([128, 8, H + 1], BF16, tag="vgm")

    # ================= stage A: projections =================
    with (
        tc.tile_pool(name="hsT", bufs=1) as hsT_pool,
        tc.tile_pool(name="wa", bufs=2) as wa_pool,
        tc.tile_pool(name="pa", space="PSUM", bufs=3) as pa_pool,
        tc.tile_pool(name="par", space="PSUM", bufs=2) as par_pool,
        tc.tile_pool(name="pat", space="PSUM", bufs=2) as pat_pool,
        tc.tile_pool(name="sa", bufs=3) as sa_pool,
    ):
        hsT = hsT_pool.tile([128, 8, TL], BF16, tag="hsT")
        hsT_r = io["hsT"].rearrange("(cc p) t -> p cc t", p=128)
        for c0, c1 in ((0, 513), (513, 1025), (1025, 1537), (1537, TL)):
            nc.sync.dma_start(hsT[:, :, c0:c1], hsT_r[:, :, c0:c1])

        kTp_v = kTp[:, :, :].rearrange("p jc (f c) -> p jc f c", c=LB)

        def global_row(whalves, bias_name):
            """projection row for the global token -> [1, C] bf16 in SBUF"""
            row = sa_pool.tile([1, C], BF16, tag="g_row")
            for jh in range(2):
                ps = par_pool.tile([1, 512], F32, tag="par")
                for cc in range(8):
                    nc.tensor.matmul(
                        ps[:], hsT[:, cc, 0:1],
                        whalves[jh][:, cc, :],
                        start=(cc == 0), stop=False)
                nc.tensor.matmul(
                    ps[:], ones1[0:1, 0:1],
                    brow[bias_name][0:1, 512 * jh:512 * jh + 512],
                    start=False, stop=True)
                nc.scalar.copy(row[0:1, 512 * jh:512 * jh + 512], ps[:])
            return row

        # ---- qT and kTp (feature-major), tokens 1..2048 ----
        for tgt in ("q", "k"):
            wnm = "wqT" if tgt == "q" else "wkT"
            bname = "bq" if tgt == "q" else "bk"
            w_r = io[wnm].rearrange("(cc p) j -> p cc j", p=128)
            whalves = []
            for w2 in range(2):
                w_sb = wa_pool.tile([128, 8, 512], BF16, tag="w")
                nc.sync.dma_start(w_sb[:], w_r[:, :, 512 * w2:512 * w2 + 512])
                whalves.append(w_sb)
                for tm in range(4):
                    t0 = 1 + 512 * tm
                    for jc2 in range(4):
                        jc = 4 * w2 + jc2
                        ps = pa_pool.tile([128, 512], F32, tag="pa")
                        for cc in range(8):
                            nc.tensor.matmul(
                                ps[:], w_sb[:, cc, 128 * jc2:128 * jc2 + 128],
                                hsT[:, cc, t0:t0 + 512],
                                start=(cc == 0), stop=(cc == 7))
                        if tgt == "q":
                            nc.vector.tensor_scalar_add(
                                qT[:, jc, t0:t0 + 512], ps[:],
                                bcol[bname][:, jc:jc + 1])
                        else:
                            nc.vector.tensor_scalar_add(
                                kTp_v[:, jc, 2 * tm:2 * tm + 2, 0:L],
                                ps[:].rearrange("p (f c) -> p f c", c=L),
                                bcol[bname][:, jc:jc + 1])
            # global token: row + transpose into feature-major column
            row = global_row(whalves, bname)
            for cc in range(8):
                pt = pat_pool.tile([128, 1], BF16, tag="pat")
                nc.tensor.transpose(pt[:], row[0:1, 128 * cc:128 * cc + 128],
                                    ident[0:1, 0:1])
                if tgt == "q":
                    nc.scalar.copy(qT[:, cc, 0:1], pt[:])
                else:
                    k0c = sa_pool.tile([128, 1], BF16, tag="k0c")
                    nc.scalar.copy(k0c[:], pt[:])
                    for f in range(NFL):
                        nc.vector.tensor_copy(kTp_v[:, cc, f, L:LB], k0c[:])

        nc.sync.dma_start(woT[:],
                          io["woT"].rearrange("(cc p) j -> p cc j", p=128))
        # ---- v (token-major), tokens 1..2048, plus v0 row ----
        w_r = io["wvT"].rearrange("(cc p) j -> p cc j", p=128)
        whalves = []
        for jh in range(2):
            w_sb = wa_pool.tile([128, 8, 512], BF16, tag="w")
            nc.sync.dma_start(w_sb[:], w_r[:, :, 512 * jh:512 * jh + 512])
            whalves.append(w_sb)
            for tn in range(16):
                t0 = 1 + 128 * tn
                ps = pa_pool.tile([128, 512], F32, tag="pa")
                for cc in range(8):
                    nc.tensor.matmul(
                        ps[:], hsT[:, cc, t0:t0 + 128],
                        w_sb[:, cc, :],
                        start=(cc == 0), stop=False)
                nc.tensor.matmul(
                    ps[:], ones1[0:1, :],
                    brow["bv"][0:1, 512 * jh:512 * jh + 512],
                    start=False, stop=True)
                nc.scalar.copy(v[:, tn, 512 * jh:512 * jh + 512], ps[:])
        row = global_row(whalves, "bv")
        nc.vector.tensor_copy(v0[:], row[:])

        # ---- Wg = blockdiag(v0) @ WoT (+ bo row at 0) ----
        nc.gpsimd.memset(vgm[:], 0.0)
        for cc in range(8):
            pt = pat_pool.tile([128, 1], BF16, tag="pat")
            nc.tensor.transpose(pt[:], v0[0:1, 128 * cc:128 * cc + 128],
                                ident[0:1, 0:1])
            vt = sa_pool.tile([128, 1], BF16, tag="vt")
            nc.scalar.copy(vt[:], pt[:])
            nc.vector.tensor_copy(vgm[0:64, cc, 1 + 2 * cc:2 + 2 * cc],
                                  vt[0:64, :])
            nc.vector.tensor_copy(vgm[64:128, cc, 2 + 2 * cc:3 + 2 * cc],
                                  vt[64:128, :])
        for jh in range(2):
            ps_wg = pa_pool.tile([H + 1, 512], F32, tag="pa")
            for cc in range(8):
                nc.tensor.matmul(ps_wg[:], vgm[:, cc, :],
                                 woT[:, cc, 512 * jh:512 * jh + 512],
                                 start=(cc == 0), stop=(cc == 7))
            nc.scalar.copy(wg[:, 512 * jh:512 * jh + 512], ps_wg[:])
        nc.vector.tensor_copy(wg[0:1, :], brow["bo"][:])

    # ---- attention-era persistent tiles (after hsT/weights freed) ----
    attn_pool = ctx.enter_context(tc.tile_pool(name="attn", bufs=1))
    OT = attn_pool.tile([128, 8, FT], BF16, tag="OT")
    Cm = attn_pool.tile([128, 16, H + 1], BF16, tag="Cm")
    nc.gpsimd.memset(Cm[:, :, 0:1], 1.0)  # ones col (idx 0) picks up bo row of Wg
    esg = attn_pool.tile([128, 16, H], BF16, tag="esg")
    qgbd = attn_pool.tile([128, 8, H], BF16, tag="qgbd")

    # ========== stage B: local frame attention + fused out-projection ==========
    with (
        tc.tile_pool(name="ps_s", space="PSUM", bufs=3) as ps_s_pool,
        tc.tile_pool(name="ps_pt", space="PSUM", bufs=2) as ps_pt_pool,
        tc.tile_pool(name="ps_o", space="PSUM", bufs=2) as ps_o_pool,
        tc.tile_pool(name="ps_y", space="PSUM", bufs=1) as ps_y_pool,
        tc.tile_pool(name="sb_p", bufs=3) as sb_p_pool,
        tc.tile_pool(name="sb_z", bufs=4) as sb_z_pool,
        tc.tile_pool(name="sb_pt", bufs=4) as sb_pt_pool,
        tc.tile_pool(name="sb_y", bufs=3) as sb_y_pool,
        tc.tile_pool(name="sb_ct", bufs=2) as sb_ct_pool,
        tc.tile_pool(name="sb_g", bufs=2) as sb_g_pool,
    ):
        nc.gpsimd.memset(qgbd[:], 0.0)
        for jc in range(8):
            nc.vector.tensor_copy(qgbd[0:64, jc, 2 * jc:2 * jc + 1],
                                  qT[0:64, jc, 0:1])
            nc.vector.tensor_copy(qgbd[64:128, jc, 2 * jc + 1:2 * jc + 2],
                                  qT[64:128, jc, 0:1])
        kTp_v = kTp[:, :, :].rearrange("p jc (f c) -> p jc f c", c=LB)

        for f in range(NFL):
            # global-query scores for this frame's two 128-token tiles
            for kc in range(2):
                tn = 2 * f + kc
                pc = LB * f + 128 * kc
                ps_sg = ps_s_pool.tile([128, H], F32, tag="ps_s")
                for jc in range(8):
                    nc.tensor.matmul(ps_sg[:], kTp[:, jc, pc:pc + 128],
                                     qgbd[:, jc, :],
                                     start=(jc == 0), stop=(jc == 7))
                nc.scalar.activation(esg[:, tn, :], ps_sg[:], AF.Exp)
            for jc in range(8):
                ps_o = ps_o_pool.tile([128, 256], F32, tag="ps_o")
                for hh in range(2):
                    h = 2 * jc + hh
                    hoff = 64 * hh
                    for qt in range(2):
                        q0 = 1 + 256 * f + 128 * qt
                        ps_s = ps_s_pool.tile([128, LB], F32, tag="ps_s")
                        nc.tensor.matmul(
                            ps_s[:],
                            qT[hoff:hoff + 64, jc, q0:q0 + 128],
                            kTp[hoff:hoff + 64, jc, LB * f:LB * f + LB],
                            start=True, stop=True)
                        pe = sb_p_pool.tile([128, LB], BF16, tag="pe")
                        z = sb_z_pool.tile([128, 1], F32, tag="z")
                        nc.scalar.activation(pe[:], ps_s[:], AF.Exp,
                                             accum_out=z[:])
                        rz = sb_z_pool.tile([128, 1], F32, tag="rz")
                        nc.vector.reciprocal(rz[:], z[:])
                        pm = sb_p_pool.tile([128, LB], BF16, tag="pm")
                        nc.gpsimd.tensor_scalar_mul(pm[:], pe[:], rz[:])
                        nc.gpsimd.tensor_copy(
                            Cm[:, 2 * f + qt, 1 + h:2 + h], pm[:, L:LB])
                        for kc in range(2):
                            ps_pt = ps_pt_pool.tile([128, 128], BF16,
                                                    tag="ps_pt")
                            nc.tensor.transpose(
                                ps_pt[:], pm[:, 128 * kc:128 * kc + 128],
                                ident[:])
                            pt = sb_pt_pool.tile([128, 128], BF16, tag="pt")
                            nc.vector.tensor_copy(pt[:], ps_pt[:])
                            nc.tensor.matmul(
                                ps_o[hoff:hoff + 64,
                                     128 * qt:128 * qt + 128],
                                v[:, 2 * f + kc,
                                  128 * jc + hoff:128 * jc + hoff + 64],
                                pt[:],
                                start=(kc == 0), stop=(kc == 1))
                c0 = 256 * f
                nc.scalar.copy(OT[:, jc, c0:c0 + 256], ps_o[:])

            # ---- out-projection for this frame's two 128-token tiles ----
            for qt in range(2):
                tn = 2 * f + qt
                ps_ct = ps_pt_pool.tile([H + 1, 128], BF16, tag="ps_pt")
                nc.tensor.transpose(ps_ct[:], Cm[:, tn, :], ident[:])
                ct = sb_ct_pool.tile([H + 1, 128], BF16, tag="ct")
                nc.vector.tensor_copy(ct[:], ps_ct[:])
                for jh in range(2):
                    ps_y = ps_y_pool.tile([128, 512], F32, tag="ps_y")
                    for jc in range(8):
                        nc.tensor.matmul(
                            ps_y[:], OT[:, jc, 128 * tn:128 * tn + 128],
                            woT[:, jc, 512 * jh:512 * jh + 512],
                            start=(jc == 0), stop=False)
                    nc.tensor.matmul(ps_y[:], ct[:],
                                     wg[:, 512 * jh:512 * jh + 512],
                                     start=False, stop=True)
                    yt = sb_y_pool.tile([128, 512], F32, tag="yt")
                    nc.scalar.copy(yt[:], ps_y[:])
                    nc.sync.dma_start(
                        io["y"][1 + 128 * tn:1 + 128 * tn + 128,
                                512 * jh:512 * jh + 512], yt[:])

        # ---- global-query: score vs global key, denominators, oG ----
        ps_sg0 = ps_s_pool.tile([1, H], F32, tag="ps_s")
        for jc in range(8):
            nc.tensor.matmul(ps_sg0[:], kTp_v[:, jc, 0, L:LB],
                             qgbd[:, jc, :],
                             start=(jc == 0), stop=(jc == 7))
        esg0 = sb_g_pool.tile([1, H], BF16, tag="esg0")
        nc.scalar.activation(esg0[:], ps_sg0[:], AF.Exp)
        esg0m = sb_g_pool.tile([1, H], BF16, tag="esg0m")
        nc.vector.tensor_scalar_mul(esg0m[:], esg0[:], gq[0:1, 0:1])

        ps_z = ps_y_pool.tile([H, 1], F32, tag="ps_y")
        for tn in range(16):
            nc.tensor.matmul(ps_z[:], esg[:, tn, :], onesc[:],
                             start=(tn == 0), stop=False)
        nc.tensor.matmul(ps_z[:], esg0m[:], onesc[0:1, :],
                         start=False, stop=True)
        nc.scalar.copy(zg[:], ps_z[:])

        for jc in range(8):
            ps_og = ps_o_pool.tile([128, 2], F32, tag="ps_o")
            for tn in range(16):
                nc.tensor.matmul(ps_og[:], v[:, tn, 128 * jc:128 * jc + 128],
                                 esg[:, tn, 2 * jc:2 * jc + 2],
                                 start=(tn == 0), stop=False)
            nc.tensor.matmul(ps_og[:], v0[0:1, 128 * jc:128 * jc + 128],
                             esg0m[0:1, 2 * jc:2 * jc + 2],
                             start=False, stop=True)
            nc.vector.tensor_copy(ogp[0:64, jc:jc + 1], ps_og[0:64, 0:1])
            nc.vector.tensor_copy(ogp[64:128, jc:jc + 1], ps_og[64:128, 1:2])
        nc.sync.dma_start(io["ogp"][:, :], ogp[:])
        nc.sync.dma_start(io["zgp"][:, :], zg[:])


_NC_CACHE = {}


def build_nc():
    if "nc" in _NC_CACHE:
        return _NC_CACHE["nc"]
    nc = bacc.Bacc("TRN2", target_bir_lowering=False, debug=False,
                   num_devices=NCORES)
    io = {
        "hsT": nc.dram_tensor("hsT", [C, TL], BF16, kind="ExternalInput").ap(),
        "wqT": nc.dram_tensor("wqT", [C, C], BF16, kind="ExternalInput").ap(),
        "wkT": nc.dram_tensor("wkT", [C, C], BF16, kind="ExternalInput").ap(),
        "wvT": nc.dram_tensor("wvT", [C, C], BF16, kind="ExternalInput").ap(),
        "woT": nc.dram_tensor("woT", [C, C], BF16, kind="ExternalInput").ap(),
        "bq": nc.dram_tensor("bq", [C], F32, kind="ExternalInput").ap(),
        "bk": nc.dram_tensor("bk", [C], F32, kind="ExternalInput").ap(),
        "bv": nc.dram_tensor("bv", [C], F32, kind="ExternalInput").ap(),
        "bo": nc.dram_tensor("bo", [C], F32, kind="ExternalInput").ap(),
        "gqm": nc.dram_tensor("gqm", [1, 1], F32, kind="ExternalInput").ap(),
        "y": nc.dram_tensor("y", [TL, C], F32, kind="ExternalOutput").ap(),
        "ogp": nc.dram_tensor("ogp", [128, 8], F32, kind="ExternalOutput").ap(),
        "zgp": nc.dram_tensor("zgp", [16, 1], F32, kind="ExternalOutput").ap(),
    }
    with tile.TileContext(nc) as tc:
        with ExitStack() as ctx:
            _emit(ctx, tc, io)
    nc.compile()
    _NC_CACHE["nc"] = nc
    return nc


def make_in_maps(hidden_states, Wq, bq, Wk, bk, Wv, bv, Wo, bo):
    """Host-side sharding: per-core input dicts."""
    hs = np.asarray(hidden_states, dtype=np.float32)
    wqT = np.ascontiguousarray((np.asarray(Wq).T * SCALE)).astype(NPBF16)
    wkT = np.ascontiguousarray(np.asarray(Wk).T).astype(NPBF16)
    wvT = np.ascontiguousarray(np.asarray(Wv).T).astype(NPBF16)
    woT = np.ascontiguousarray(np.asarray(Wo).T).astype(NPBF16)
    bq_s = (np.asarray(bq, np.float32) * SCALE).astype(np.float32)
    bk_ = np.asarray(bk, np.float32)
    bv_ = np.asarray(bv, np.float32)
    bo_ = np.asarray(bo, np.float32)

    in_maps = []
    for core in range(NCORES):
        b, th = core // 2, core % 2
        hsb = hs[b]  # [T, C]
        sl = hsb[1 + th * FT: 1 + (th + 1) * FT]  # [2048, C]
        shard = np.concatenate([hsb[0:1], sl], axis=0)  # [2049, C]
        hsT = np.ascontiguousarray(shard.T).astype(NPBF16)  # [C, 2049]
        in_maps.append({
            "hsT": hsT,
            "wqT": wqT, "wkT": wkT, "wvT": wvT, "woT": woT,
            "bq": bq_s, "bk": bk_, "bv": bv_, "bo": bo_,
            "gqm": np.array([[1.0 if th == 0 else 0.0]], np.float32),
        })
    return in_maps


def assemble(results, Wo, bo):
    """Host-side gather: concat per-core halves; combine global-query row."""
    Wo = np.asarray(Wo, np.float32)
    bo_ = np.asarray(bo, np.float32)
    out = np.empty((B, T, C), np.float32)
    for b in range(B):
        r0, r1 = results[2 * b], results[2 * b + 1]
        out[b, 1:1 + FT] = r0["y"][1:]
        out[b, 1 + FT:] = r1["y"][1:]
        og = r0["ogp"] + r1["ogp"]          # [128, 8]
        zgs = (r0["zgp"] + r1["zgp"])[:, 0]  # [16]
        vec = np.empty((C,), np.float32)
        for h in range(H):
            jc, hoff = h // 2, 64 * (h % 2)
            vec[h * DH:(h + 1) * DH] = og[hoff:hoff + 64, jc] / zgs[h]
        out[b, 0] = vec @ Wo.T + bo_
    return out


_LAST_RESULTS = {}


def kernel(hidden_states, Wq, bq, Wk, bk, Wv, bv, Wo, bo, M, N, L,
           trace=False):
    assert int(M) == 1 and int(N) == 16 and int(L) == 256
    nc = build_nc()
    in_maps = make_in_maps(hidden_states, Wq, bq, Wk, bk, Wv, bv, Wo, bo)
    res = run_bass_kernel_spmd(nc, in_maps, core_ids=list(range(NCORES)),
                               trace=trace)
    _LAST_RESULTS["res"] = res
    return assemble(res.results, Wo, bo)


# revision 25
# speedup vs baseline: 204.0394x; 204.0394x over previous
"""CLIPAttention sparse-attention kernel for 8 Trainium2 NeuronCores.

Problem shapes: B=4, T=4097 (1 global token + 16 frames x 256), C=1024, H=16.

Sharding: core = b*2 + th  (b = batch 0..3, th = token-half 0..1).
Each core handles ALL 16 heads for its 2048 frame tokens (8 frames) plus a
copy of the global token.  The only cross-core coupling is the single global
query (token 0), handled by unnormalized partial sums (oG, zG) that the host
combines (a ~4KB reduce + one 1024x1024 matvec per batch).

Device-side per core:
  stage A: hsT [C, 2049] (bf16, global token at local col 0) -> qT, kT
           (feature-major, kT in a padded per-frame layout with the global
           key replicated at col 256 of each 257-wide frame block), v
           (token-major), q0/k0/v0 rows for the global token, and
           Wg = blockdiag(v0) @ WoT with bo in row 0 (picked up by the
           C-matrix ones column).
  frame loop: per (frame, head, query-tile): S = qT^T kT_block [128q, 257]
           (bf16 matmul, fp32 PSUM), exp on ACT with accum_out giving the
           softmax denominator (no max subtraction - scores are O(1) by
           construction), reciprocal+normalize on DVE, PE-transpose the two
           128-wide local key columns, PV matmuls write O^T at the head's
           partition offset; normalized global-key weight lands in column
           1+h of C [128q, 1+H].  After each frame: the fused out-projection
           Y = OT^T WoT + C^T Wg for the frame's two 128-token tiles.
  tail:    global-query partials batched over all tokens via block-diagonal
           q0 rhs (sGT [128t, 16]), exp, denominators via ones-matmul, oG
           via paired-head matmuls with junk-half extraction.
"""

import numpy as np
import ml_dtypes
from contextlib import ExitStack

import concourse.bass as bass
import concourse.bacc as bacc
import concourse.tile as tile
import concourse.mybir as mybir
from concourse.bass_utils import run_bass_kernel_spmd
from concourse.masks import make_identity

BF16 = mybir.dt.bfloat16
F32 = mybir.dt.float32
AF = mybir.ActivationFunctionType
NPBF16 = ml_dtypes.bfloat16

B, T, C, H, DH = 4, 4097, 1024, 16, 64
NFRAME, L = 16, 256  # total frames, frame length
SCALE = DH ** -0.5
FT = 2048            # frame tokens per core
TL = FT + 1          # local tokens (global token at local col 0)
NFL = 8              # frames per core
NCORES = 8
LB = L + 1           # padded key block: 256 frame keys + 1 global key


def _emit(ctx: ExitStack, tc, io):
    nc = tc.nc

    # ---------------- constant / small tiles ----------------
    const_pool = ctx.enter_context(tc.tile_pool(name="const", bufs=1))
    ident = const_pool.tile([128, 128], BF16, tag="ident")
    make_identity(nc, ident[:])
    ones1 = const_pool.tile([1, 128], BF16, tag="ones1")
    nc.gpsimd.memset(ones1[:], 1.0)
    onesc = const_pool.tile([128, 1], BF16, tag="onesc")
    nc.gpsimd.memset(onesc[:], 1.0)

    gq = const_pool.tile([1, 1], F32, tag="gq")
    bcol = {}
    brow = {}

    # ---------------- persistent big tiles ----------------
    big_pool = ctx.enter_context(tc.tile_pool(name="big", bufs=1))
    qT = big_pool.tile([128, 8, TL], BF16, tag="qT")
    kTp = big_pool.tile([128, 8, NFL * LB], BF16, tag="kTp")
    v = big_pool.tile([128, 16, C], BF16, tag="v")
    v0 = big_pool.tile([1, C], BF16, tag="v0")
    ogp = big_pool.tile([128, 8], F32, tag="ogp")
    zg = big_pool.tile([16, 1], F32, tag="zg")
    woT = big_pool.tile([128, 8, C], BF16, tag="woT")
    wg = big_pool.tile([H + 1, C], BF16, tag="wg")
    vgm = big_pool.tile([128, 8, H + 1], BF16, tag="vgm")

    # ================= stage A: projections =================
    with (
        tc.tile_pool(name="hsT", bufs=1) as hsT_pool,
        tc.tile_pool(name="wa", bufs=2) as wa_pool,
        tc.tile_pool(name="pa", space="PSUM", bufs=3) as pa_pool,
        tc.tile_pool(name="par", space="PSUM", bufs=2) as par_pool,
        tc.tile_pool(name="pat", space="PSUM", bufs=2) as pat_pool,
        tc.tile_pool(name="sa", bufs=3) as sa_pool,
    ):
        hsT = hsT_pool.tile([128, 8, TL], BF16, tag="hsT")
        hsT_r = io["hsT"].rearrange("(cc p) t -> p cc t", p=128)
        for c0, c1 in ((0, 513), (513, 1025), (1025, 1537), (1537, TL)):
            nc.sync.dma_start(hsT[:, :, c0:c1], hsT_r[:, :, c0:c1])

        nc.sync.dma_start(gq[:], io["gqm"][:, :])
        with tc.tile_pool(name="brf", bufs=2) as brf_pool:
            for nm in ("bq", "bk", "bv", "bo"):
                t = const_pool.tile([128, 8], F32, tag=f"{nm}c")
                nc.sync.dma_start(t[:],
                                  io[nm].rearrange("(jc p) -> p jc", p=128))
                bcol[nm] = t
                rf = brf_pool.tile([1, C], F32, tag="brf")
                nc.sync.dma_start(rf[:], io[nm].rearrange("(a j) -> a j", a=1))
                rb = const_pool.tile([1, C], BF16, tag=f"{nm}r")
                nc.scalar.copy(rb[:], rf[:])
                brow[nm] = rb

        kTp_v = kTp[:, :, :].rearrange("p jc (f c) -> p jc f c", c=LB)

        def global_row(whalves, bias_name):
            """projection row for the global token -> [1, C] bf16 in SBUF"""
            row = sa_pool.tile([1, C], BF16, tag="g_row")
            for jh in range(2):
                ps = par_pool.tile([1, 512], F32, tag="par")
                for cc in range(8):
                    nc.tensor.matmul(
                        ps[:], hsT[:, cc, 0:1],
                        whalves[jh][:, cc, :],
                        start=(cc == 0), stop=False)
                nc.tensor.matmul(
                    ps[:], ones1[0:1, 0:1],
                    brow[bias_name][0:1, 512 * jh:512 * jh + 512],
                    start=False, stop=True)
                nc.scalar.copy(row[0:1, 512 * jh:512 * jh + 512], ps[:])
            return row

        # ---- qT and kTp (feature-major), tokens 1..2048 ----
        for tgt in ("q", "k"):
            wnm = "wqT" if tgt == "q" else "wkT"
            bname = "bq" if tgt == "q" else "bk"
            w_r = io[wnm].rearrange("(cc p) j -> p cc j", p=128)
            whalves = []
            for w2 in range(2):
                w_sb = wa_pool.tile([128, 8, 512], BF16, tag="w")
                nc.sync.dma_start(w_sb[:], w_r[:, :, 512 * w2:512 * w2 + 512])
                whalves.append(w_sb)
                for tm in range(4):
                    t0 = 1 + 512 * tm
                    for jc2 in range(4):
                        jc = 4 * w2 + jc2
                        ps = pa_pool.tile([128, 512], F32, tag="pa")
                        for cc in range(8):
                            nc.tensor.matmul(
                                ps[:], w_sb[:, cc, 128 * jc2:128 * jc2 + 128],
                                hsT[:, cc, t0:t0 + 512],
                                start=(cc == 0), stop=(cc == 7))
                        if tgt == "q":
                            nc.vector.tensor_scalar_add(
                                qT[:, jc, t0:t0 + 512], ps[:],
                                bcol[bname][:, jc:jc + 1])
                        else:
                            nc.vector.tensor_scalar_add(
                                kTp_v[:, jc, 2 * tm:2 * tm + 2, 0:L],
                                ps[:].rearrange("p (f c) -> p f c", c=L),
                                bcol[bname][:, jc:jc + 1])
            # global token: row + transpose into feature-major column
            row = global_row(whalves, bname)
            for cc in range(8):
                pt = pat_pool.tile([128, 1], BF16, tag="pat")
                nc.tensor.transpose(pt[:], row[0:1, 128 * cc:128 * cc + 128],
                                    ident[0:1, 0:1])
                if tgt == "q":
                    nc.scalar.copy(qT[:, cc, 0:1], pt[:])
                else:
                    k0c = sa_pool.tile([128, 1], BF16, tag="k0c")
                    nc.scalar.copy(k0c[:], pt[:])
                    for f in range(NFL):
                        nc.vector.tensor_copy(kTp_v[:, cc, f, L:LB], k0c[:])

        nc.sync.dma_start(woT[:],
                          io["woT"].rearrange("(cc p) j -> p cc j", p=128))
        # ---- v (token-major), tokens 1..2048, plus v0 row ----
        w_r = io["wvT"].rearrange("(cc p) j -> p cc j", p=128)
        whalves = []
        for jh in range(2):
            w_sb = wa_pool.tile([128, 8, 512], BF16, tag="w")
            nc.sync.dma_start(w_sb[:], w_r[:, :, 512 * jh:512 * jh + 512])
            whalves.append(w_sb)
            for tn in range(16):
                t0 = 1 + 128 * tn
                ps = pa_pool.tile([128, 512], F32, tag="pa")
                for cc in range(8):
                    nc.tensor.matmul(
                        ps[:], hsT[:, cc, t0:t0 + 128],
                        w_sb[:, cc, :],
                        start=(cc == 0), stop=False)
                nc.tensor.matmul(
                    ps[:], ones1[0:1, :],
                    brow["bv"][0:1, 512 * jh:512 * jh + 512],
                    start=False, stop=True)
                nc.scalar.copy(v[:, tn, 512 * jh:512 * jh + 512], ps[:])
        row = global_row(whalves, "bv")
        nc.vector.tensor_copy(v0[:], row[:])

        # ---- Wg = blockdiag(v0) @ WoT (+ bo row at 0) ----
        nc.gpsimd.memset(vgm[:], 0.0)
        for cc in range(8):
            pt = pat_pool.tile([128, 1], BF16, tag="pat")
            nc.tensor.transpose(pt[:], v0[0:1, 128 * cc:128 * cc + 128],
                                ident[0:1, 0:1])
            vt = sa_pool.tile([128, 1], BF16, tag="vt")
            nc.scalar.copy(vt[:], pt[:])
            nc.vector.tensor_copy(vgm[0:64, cc, 1 + 2 * cc:2 + 2 * cc],
                                  vt[0:64, :])
            nc.vector.tensor_copy(vgm[64:128, cc, 2 + 2 * cc:3 + 2 * cc],
                                  vt[64:128, :])
        for jh in range(2):
            ps_wg = pa_pool.tile([H + 1, 512], F32, tag="pa")
            for cc in range(8):
                nc.tensor.matmul(ps_wg[:], vgm[:, cc, :],
                                 woT[:, cc, 512 * jh:512 * jh + 512],
                                 start=(cc == 0), stop=(cc == 7))
            nc.scalar.copy(wg[:, 512 * jh:512 * jh + 512], ps_wg[:])
        nc.vector.tensor_copy(wg[0:1, :], brow["bo"][:])

    # ---- attention-era persistent tiles (after hsT/weights freed) ----
    attn_pool = ctx.enter_context(tc.tile_pool(name="attn", bufs=1))
    OT = attn_pool.tile([128, 8, FT], BF16, tag="OT")
    Cm = attn_pool.tile([128, 16, H + 1], BF16, tag="Cm")
    nc.gpsimd.memset(Cm[:, :, 0:1], 1.0)  # ones col (idx 0) picks up bo row of Wg
    esg = attn_pool.tile([128, 16, H], BF16, tag="esg")
    qgbd = attn_pool.tile([128, 8, H], BF16, tag="qgbd")

    # ========== stage B: local frame attention + fused out-projection ==========
    with (
        tc.tile_pool(name="ps_s", space="PSUM", bufs=3) as ps_s_pool,
        tc.tile_pool(name="ps_pt", space="PSUM", bufs=2) as ps_pt_pool,
        tc.tile_pool(name="ps_o", space="PSUM", bufs=2) as ps_o_pool,
        tc.tile_pool(name="ps_y", space="PSUM", bufs=1) as ps_y_pool,
        tc.tile_pool(name="sb_p", bufs=5) as sb_p_pool,
        tc.tile_pool(name="sb_z", bufs=6) as sb_z_pool,
        tc.tile_pool(name="sb_pt", bufs=6) as sb_pt_pool,
        tc.tile_pool(name="sb_y", bufs=3) as sb_y_pool,
        tc.tile_pool(name="sb_ct", bufs=2) as sb_ct_pool,
        tc.tile_pool(name="sb_g", bufs=2) as sb_g_pool,
    ):
        nc.gpsimd.memset(qgbd[:], 0.0)
        for jc in range(8):
            nc.vector.tensor_copy(qgbd[0:64, jc, 2 * jc:2 * jc + 1],
                                  qT[0:64, jc, 0:1])
            nc.vector.tensor_copy(qgbd[64:128, jc, 2 * jc + 1:2 * jc + 2],
                                  qT[64:128, jc, 0:1])
        kTp_v = kTp[:, :, :].rearrange("p jc (f c) -> p jc f c", c=LB)

        for f in range(NFL):
            for jc in range(8):
                ps_o = ps_o_pool.tile([128, 256], F32, tag="ps_o")
                for hh in range(2):
                    h = 2 * jc + hh
                    hoff = 64 * hh
                    for qt in range(2):
                        q0 = 1 + 256 * f + 128 * qt
                        ps_s = ps_s_pool.tile([128, LB], F32, tag="ps_s")
                        nc.tensor.matmul(
                            ps_s[:],
                            qT[hoff:hoff + 64, jc, q0:q0 + 128],
                            kTp[hoff:hoff + 64, jc, LB * f:LB * f + LB],
                            start=True, stop=True)
                        pe = sb_p_pool.tile([128, LB], BF16, tag="pe")
                        z = sb_z_pool.tile([128, 1], F32, tag="z")
                        nc.scalar.activation(pe[:], ps_s[:], AF.Exp,
                                             accum_out=z[:])
                        rz = sb_z_pool.tile([128, 1], F32, tag="rz")
                        nc.vector.reciprocal(rz[:], z[:])
                        pm = sb_p_pool.tile([128, LB], BF16, tag="pm")
                        nc.vector.tensor_scalar_mul(pm[:], pe[:], rz[:])
                        nc.vector.tensor_copy(
                            Cm[:, 2 * f + qt, 1 + h:2 + h], pm[:, L:LB])
                        for kc in range(2):
                            ps_pt = ps_pt_pool.tile([128, 128], BF16,
                                                    tag="ps_pt")
                            nc.tensor.transpose(
                                ps_pt[:], pm[:, 128 * kc:128 * kc + 128],
                                ident[:])
                            pt = sb_pt_pool.tile([128, 128], BF16, tag="pt")
                            nc.vector.tensor_copy(pt[:], ps_pt[:])
                            nc.tensor.matmul(
                                ps_o[hoff:hoff + 64,
                                     128 * qt:128 * qt + 128],
                                v[:, 2 * f + kc,
                                  128 * jc + hoff:128 * jc + hoff + 64],
                                pt[:],
                                start=(kc == 0), stop=(kc == 1))
                c0 = 256 * f
                nc.scalar.copy(OT[:, jc, c0:c0 + 256], ps_o[:])

            # ---- out-projection for this frame's two 128-token tiles ----
            for qt in range(2):
                tn = 2 * f + qt
                ps_ct = ps_pt_pool.tile([H + 1, 128], BF16, tag="ps_pt")
                nc.tensor.transpose(ps_ct[:], Cm[:, tn, :], ident[:])
                ct = sb_ct_pool.tile([H + 1, 128], BF16, tag="ct")
                nc.vector.tensor_copy(ct[:], ps_ct[:])
                for jh in range(2):
                    ps_y = ps_y_pool.tile([128, 512], F32, tag="ps_y")
                    for jc in range(8):
                        nc.tensor.matmul(
                            ps_y[:], OT[:, jc, 128 * tn:128 * tn + 128],
                            woT[:, jc, 512 * jh:512 * jh + 512],
                            start=(jc == 0), stop=False)
                    nc.tensor.matmul(ps_y[:], ct[:],
                                     wg[:, 512 * jh:512 * jh + 512],
                                     start=False, stop=True)
                    yt = sb_y_pool.tile([128, 512], F32, tag="yt")
                    nc.vector.tensor_copy(yt[:], ps_y[:])
                    nc.sync.dma_start(
                        io["y"][1 + 128 * tn:1 + 128 * tn + 128,
                                512 * jh:512 * jh + 512], yt[:])

        # ---- global-query: scores, denominators, oG ----
        for tn in range(16):
            f, kc = tn // 2, tn % 2
            pc = LB * f + 128 * kc
            ps_sg = ps_s_pool.tile([128, H], F32, tag="ps_s")
            for jc in range(8):
                nc.tensor.matmul(ps_sg[:], kTp[:, jc, pc:pc + 128],
                                 qgbd[:, jc, :],
                                 start=(jc == 0), stop=(jc == 7))
            nc.scalar.activation(esg[:, tn, :], ps_sg[:], AF.Exp)
        ps_sg0 = ps_s_pool.tile([1, H], F32, tag="ps_s")
        for jc in range(8):
            nc.tensor.matmul(ps_sg0[:], kTp_v[:, jc, 0, L:LB],
                             qgbd[:, jc, :],
                             start=(jc == 0), stop=(jc == 7))
        esg0 = sb_g_pool.tile([1, H], BF16, tag="esg0")
        nc.scalar.activation(esg0[:], ps_sg0[:], AF.Exp)
        esg0m = sb_g_pool.tile([1, H], BF16, tag="esg0m")
        nc.vector.tensor_scalar_mul(esg0m[:], esg0[:], gq[0:1, 0:1])

        ps_z = ps_y_pool.tile([H, 1], F32, tag="ps_y")
        for tn in range(16):
            nc.tensor.matmul(ps_z[:], esg[:, tn, :], onesc[:],
                             start=(tn == 0), stop=False)
        nc.tensor.matmul(ps_z[:], esg0m[:], onesc[0:1, :],
                         start=False, stop=True)
        nc.scalar.copy(zg[:], ps_z[:])

        for jc in range(8):
            ps_og = ps_o_pool.tile([128, 2], F32, tag="ps_o")
            for tn in range(16):
                nc.tensor.matmul(ps_og[:], v[:, tn, 128 * jc:128 * jc + 128],
                                 esg[:, tn, 2 * jc:2 * jc + 2],
                                 start=(tn == 0), stop=False)
            nc.tensor.matmul(ps_og[:], v0[0:1, 128 * jc:128 * jc + 128],
                             esg0m[0:1, 2 * jc:2 * jc + 2],
                             start=False, stop=True)
            nc.vector.tensor_copy(ogp[0:64, jc:jc + 1], ps_og[0:64, 0:1])
            nc.vector.tensor_copy(ogp[64:128, jc:jc + 1], ps_og[64:128, 1:2])
        nc.sync.dma_start(io["ogp"][:, :], ogp[:])
        nc.sync.dma_start(io["zgp"][:, :], zg[:])


_NC_CACHE = {}


def build_nc():
    if "nc" in _NC_CACHE:
        return _NC_CACHE["nc"]
    nc = bacc.Bacc("TRN2", target_bir_lowering=False, debug=False,
                   num_devices=NCORES)
    io = {
        "hsT": nc.dram_tensor("hsT", [C, TL], BF16, kind="ExternalInput").ap(),
        "wqT": nc.dram_tensor("wqT", [C, C], BF16, kind="ExternalInput").ap(),
        "wkT": nc.dram_tensor("wkT", [C, C], BF16, kind="ExternalInput").ap(),
        "wvT": nc.dram_tensor("wvT", [C, C], BF16, kind="ExternalInput").ap(),
        "woT": nc.dram_tensor("woT", [C, C], BF16, kind="ExternalInput").ap(),
        "bq": nc.dram_tensor("bq", [C], F32, kind="ExternalInput").ap(),
        "bk": nc.dram_tensor("bk", [C], F32, kind="ExternalInput").ap(),
        "bv": nc.dram_tensor("bv", [C], F32, kind="ExternalInput").ap(),
        "bo": nc.dram_tensor("bo", [C], F32, kind="ExternalInput").ap(),
        "gqm": nc.dram_tensor("gqm", [1, 1], F32, kind="ExternalInput").ap(),
        "y": nc.dram_tensor("y", [TL, C], F32, kind="ExternalOutput").ap(),
        "ogp": nc.dram_tensor("ogp", [128, 8], F32, kind="ExternalOutput").ap(),
        "zgp": nc.dram_tensor("zgp", [16, 1], F32, kind="ExternalOutput").ap(),
    }
    with tile.TileContext(nc) as tc:
        with ExitStack() as ctx:
            _emit(ctx, tc, io)
    nc.compile()
    _NC_CACHE["nc"] = nc
    return nc


def make_in_maps(hidden_states, Wq, bq, Wk, bk, Wv, bv, Wo, bo):
    """Host-side sharding: per-core input dicts."""
    hs = np.asarray(hidden_states, dtype=np.float32)
    wqT = np.ascontiguousarray((np.asarray(Wq).T * SCALE)).astype(NPBF16)
    wkT = np.ascontiguousarray(np.asarray(Wk).T).astype(NPBF16)
    wvT = np.ascontiguousarray(np.asarray(Wv).T).astype(NPBF16)
    woT = np.ascontiguousarray(np.asarray(Wo).T).astype(NPBF16)
    bq_s = (np.asarray(bq, np.float32) * SCALE).astype(np.float32)
    bk_ = np.asarray(bk, np.float32)
    bv_ = np.asarray(bv, np.float32)
    bo_ = np.asarray(bo, np.float32)

    in_maps = []
    for core in range(NCORES):
        b, th = core // 2, core % 2
        hsb = hs[b]  # [T, C]
        sl = hsb[1 + th * FT: 1 + (th + 1) * FT]  # [2048, C]
        shard = np.concatenate([hsb[0:1], sl], axis=0)  # [2049, C]
        hsT = np.ascontiguousarray(shard.T).astype(NPBF16)  # [C, 2049]
        in_maps.append({
            "hsT": hsT,
            "wqT": wqT, "wkT": wkT, "wvT": wvT, "woT": woT,
            "bq": bq_s, "bk": bk_, "bv": bv_, "bo": bo_,
            "gqm": np.array([[1.0 if th == 0 else 0.0]], np.float32),
        })
    return in_maps


def assemble(results, Wo, bo):
    """Host-side gather: concat per-core halves; combine global-query row."""
    Wo = np.asarray(Wo, np.float32)
    bo_ = np.asarray(bo, np.float32)
    out = np.empty((B, T, C), np.float32)
    for b in range(B):
        r0, r1 = results[2 * b], results[2 * b + 1]
        out[b, 1:1 + FT] = r0["y"][1:]
        out[b, 1 + FT:] = r1["y"][1:]
        og = r0["ogp"] + r1["ogp"]          # [128, 8]
        zgs = (r0["zgp"] + r1["zgp"])[:, 0]  # [16]
        vec = np.empty((C,), np.float32)
        for h in range(H):
            jc, hoff = h // 2, 64 * (h % 2)
            vec[h * DH:(h + 1) * DH] = og[hoff:hoff + 64, jc] / zgs[h]
        out[b, 0] = vec @ Wo.T + bo_
    return out


_LAST_RESULTS = {}


def kernel(hidden_states, Wq, bq, Wk, bk, Wv, bv, Wo, bo, M, N, L,
           trace=False):
    assert int(M) == 1 and int(N) == 16 and int(L) == 256
    nc = build_nc()
    in_maps = make_in_maps(hidden_states, Wq, bq, Wk, bk, Wv, bv, Wo, bo)
    res = run_bass_kernel_spmd(nc, in_maps, core_ids=list(range(NCORES)),
                               trace=trace)
    _LAST_RESULTS["res"] = res
    return assemble(res.results, Wo, bo)


# revision 30
# speedup vs baseline: 234.8884x; 1.1512x over previous
"""CLIPAttention sparse-attention kernel for 8 Trainium2 NeuronCores.

Problem shapes: B=4, T=4097 (1 global token + 16 frames x 256), C=1024, H=16.

Sharding: core = b*2 + th  (b = batch 0..3, th = token-half 0..1).
Each core handles ALL 16 heads for its 2048 frame tokens (8 frames) plus a
copy of the global token.  The only cross-core coupling is the single global
query (token 0), handled by unnormalized partial sums (oG, zG) that the host
combines (a ~4KB reduce + one 1024x1024 matvec per batch).

Device-side per core:
  stage A: hsT [C, 2049] (bf16, global token at local col 0) -> qT, kT
           (feature-major, kT in a padded per-frame layout with the global
           key replicated at col 256 of each 257-wide frame block), v
           (token-major), q0/k0/v0 rows for the global token, and
           Wg = blockdiag(v0) @ WoT with bo in row 0 (picked up by the
           C-matrix ones column).
  frame loop: per (frame, head, query-tile): S = qT^T kT_block [128q, 257]
           (bf16 matmul, fp32 PSUM), exp on ACT with accum_out giving the
           softmax denominator (no max subtraction - scores are O(1) by
           construction), reciprocal+normalize on DVE, PE-transpose the two
           128-wide local key columns, PV matmuls write O^T at the head's
           partition offset; normalized global-key weight lands in column
           1+h of C [128q, 1+H].  After each frame: the fused out-projection
           Y = OT^T WoT + C^T Wg for the frame's two 128-token tiles.
  tail:    global-query partials batched over all tokens via block-diagonal
           q0 rhs (sGT [128t, 16]), exp, denominators via ones-matmul, oG
           via paired-head matmuls with junk-half extraction.
"""

import numpy as np
import ml_dtypes
from contextlib import ExitStack

import concourse.bass as bass
import concourse.bacc as bacc
import concourse.tile as tile
import concourse.mybir as mybir
from concourse.bass_utils import run_bass_kernel_spmd
from concourse.masks import make_identity

BF16 = mybir.dt.bfloat16
F32 = mybir.dt.float32
AF = mybir.ActivationFunctionType
NPBF16 = ml_dtypes.bfloat16

B, T, C, H, DH = 4, 4097, 1024, 16, 64
NFRAME, L = 16, 256  # total frames, frame length
SCALE = DH ** -0.5
FT = 2048            # frame tokens per core
TL = FT + 1          # local tokens (global token at local col 0)
NFL = 8              # frames per core
NCORES = 8
LB = L + 1           # padded key block: 256 frame keys + 1 global key


def _emit(ctx: ExitStack, tc, io):
    nc = tc.nc

    # ---------------- constant / small tiles ----------------
    const_pool = ctx.enter_context(tc.tile_pool(name="const", bufs=1))
    ident = const_pool.tile([128, 128], BF16, tag="ident")
    make_identity(nc, ident[:])
    ones1 = const_pool.tile([1, 128], BF16, tag="ones1")
    nc.gpsimd.memset(ones1[:], 1.0)
    onesc = const_pool.tile([128, 1], BF16, tag="onesc")
    nc.gpsimd.memset(onesc[:], 1.0)
    gq = const_pool.tile([1, 1], F32, tag="gq")

    # ---------------- persistent tiles ----------------
    big_pool = ctx.enter_context(tc.tile_pool(name="big", bufs=1))
    woT = big_pool.tile([128, 8, C], BF16, tag="woT")
    wg = big_pool.tile([H + 1, C], BF16, tag="wg")
    vgm = big_pool.tile([128, 8, H + 1], BF16, tag="vgm")
    v0 = big_pool.tile([1, C], BF16, tag="v0")
    k0 = big_pool.tile([128, 8], BF16, tag="k0")
    qgbd = big_pool.tile([128, 8, H], BF16, tag="qgbd")
    OT = big_pool.tile([128, 8, FT], BF16, tag="OT")
    Cm = big_pool.tile([128, 16, H + 1], BF16, tag="Cm")
    esg = big_pool.tile([128, 16, H], BF16, tag="esg")
    esg0m = big_pool.tile([1, H], BF16, tag="esg0m")
    og_acc = big_pool.tile([128, 2 * H], F32, tag="og_acc")
    ogp = big_pool.tile([128, 8], F32, tag="ogp")
    zg = big_pool.tile([16, 1], F32, tag="zg")
    nc.gpsimd.memset(Cm[:, :, 0:1], 1.0)   # ones col picks up bo row of Wg
    nc.gpsimd.memset(qgbd[:], 0.0)
    nc.gpsimd.memset(og_acc[:], 0.0)

    hs_pool = ctx.enter_context(tc.tile_pool(name="hsr", bufs=2))
    wa_pool = ctx.enter_context(tc.tile_pool(name="wa", bufs=6))
    q_pool = ctx.enter_context(tc.tile_pool(name="qr", bufs=2))
    k_pool = ctx.enter_context(tc.tile_pool(name="kr", bufs=2))
    v_pool = ctx.enter_context(tc.tile_pool(name="vr", bufs=2))
    pa_pool = ctx.enter_context(tc.tile_pool(name="pa", space="PSUM", bufs=2))
    ps_s_pool = ctx.enter_context(tc.tile_pool(name="ps_s", space="PSUM", bufs=2))
    ps_pt_pool = ctx.enter_context(tc.tile_pool(name="ps_pt", space="PSUM", bufs=2))
    ps_o_pool = ctx.enter_context(tc.tile_pool(name="ps_o", space="PSUM", bufs=1))
    ps_y_pool = ctx.enter_context(tc.tile_pool(name="ps_y", space="PSUM", bufs=1))
    sa_pool = ctx.enter_context(tc.tile_pool(name="sa", bufs=1))
    sg3_pool = ctx.enter_context(tc.tile_pool(name="sg3", bufs=3))
    sb_p_pool = ctx.enter_context(tc.tile_pool(name="sb_p", bufs=4))
    sb_z_pool = ctx.enter_context(tc.tile_pool(name="sb_z", bufs=6))
    sb_pt_pool = ctx.enter_context(tc.tile_pool(name="sb_pt", bufs=6))
    sb_y_pool = ctx.enter_context(tc.tile_pool(name="sb_y", bufs=2))
    sb_ct_pool = ctx.enter_context(tc.tile_pool(name="sb_ct", bufs=2))
    if True:
        hsT_r = io["hsT"].rearrange("(cc p) t -> p cc t", p=128)
        hs0 = sa_pool.tile([128, 8, 1], BF16, tag="hs0")
        nc.sync.dma_start(hs0[:], hsT_r[:, :, 0:1])

        # weights: six [128, 8, 512] half-tiles, resident throughout.
        # First hs chunk is queued right after wq so tm=0 compute starts
        # without waiting for the remaining ~20MB of weight DMA.
        hs_tiles = {}

        def get_hs(tm):
            if tm not in hs_tiles:
                t = hs_pool.tile([128, 8, 512], BF16, tag="hs")
                nc.sync.dma_start(
                    t[:], hsT_r[:, :, 1 + 512 * tm:1 + 512 * tm + 512])
                hs_tiles[tm] = t
            return hs_tiles[tm]

        wh = {}
        for wnm, key in (("wqT", "q"), ("wkT", "k"), ("wvT", "v")):
            w_r = io[wnm].rearrange("(cc p) j -> p cc j", p=128)
            for w2 in range(2):
                t = wa_pool.tile([128, 8, 512], BF16, tag="w")
                nc.sync.dma_start(t[:], w_r[:, :, 512 * w2:512 * w2 + 512])
                wh[key, w2] = t
            if key == "q":
                get_hs(0)
        nc.sync.dma_start(gq[:], io["gqm"][:, :])

        bcol = {}
        brow = {}
        with tc.tile_pool(name="brf", bufs=2) as brf_pool:
            for nm in ("bq", "bk", "bv", "bo"):
                t = const_pool.tile([128, 8], F32, tag=f"{nm}c")
                nc.sync.dma_start(t[:],
                                  io[nm].rearrange("(jc p) -> p jc", p=128))
                bcol[nm] = t
            for nm in ("bv", "bo"):
                rf = brf_pool.tile([1, C], F32, tag="brf")
                nc.sync.dma_start(rf[:], io[nm].rearrange("(a j) -> a j", a=1))
                rb = const_pool.tile([1, C], BF16, tag=f"{nm}r")
                nc.scalar.copy(rb[:], rf[:])
                brow[nm] = rb
        nc.sync.dma_start(woT[:],
                          io["woT"].rearrange("(cc p) j -> p cc j", p=128))

        def global_row(key, bias_row):
            """projection row of the global token -> [1, C] bf16"""
            row = sg3_pool.tile([1, C], BF16, tag="g_row")
            for jh in range(2):
                ps = pa_pool.tile([1, 512], F32, tag="pa")
                for cc in range(8):
                    nc.tensor.matmul(ps[:], hs0[:, cc, :],
                                     wh[key, jh][:, cc, :],
                                     start=(cc == 0), stop=False)
                nc.tensor.matmul(ps[:], ones1[0:1, 0:1],
                                 bias_row[0:1, 512 * jh:512 * jh + 512],
                                 start=False, stop=True)
                nc.scalar.copy(row[0:1, 512 * jh:512 * jh + 512], ps[:])
            return row

        zrow = const_pool.tile([1, C], BF16, tag="zrow")
        nc.gpsimd.memset(zrow[:], 0.0)

        # ---- global-token projections, qgbd/k0, Wg (once, up front) ----
        q0row = global_row("q", zrow)   # bias added after transpose (bcol)
        k0row = global_row("k", zrow)
        v0row = global_row("v", brow["bv"])
        nc.vector.tensor_copy(v0[:], v0row[:])
        nc.gpsimd.memset(vgm[:], 0.0)
        for cc in range(8):
            pt = ps_pt_pool.tile([128, 1], BF16, tag="ps_pt")
            nc.tensor.transpose(pt[:], q0row[0:1, 128 * cc:128 * cc + 128],
                                ident[0:1, 0:1])
            nc.vector.tensor_scalar_add(
                qgbd[0:64, cc, 2 * cc:2 * cc + 1], pt[0:64, :],
                bcol["bq"][0:64, cc:cc + 1])
            nc.vector.tensor_scalar_add(
                qgbd[64:128, cc, 2 * cc + 1:2 * cc + 2], pt[64:128, :],
                bcol["bq"][64:128, cc:cc + 1])
            pt = ps_pt_pool.tile([128, 1], BF16, tag="ps_pt")
            nc.tensor.transpose(pt[:], k0row[0:1, 128 * cc:128 * cc + 128],
                                ident[0:1, 0:1])
            nc.vector.tensor_scalar_add(k0[:, cc:cc + 1], pt[:],
                                        bcol["bk"][:, cc:cc + 1])
            pt = ps_pt_pool.tile([128, 1], BF16, tag="ps_pt")
            nc.tensor.transpose(pt[:], v0row[0:1, 128 * cc:128 * cc + 128],
                                ident[0:1, 0:1])
            vt = sb_z_pool.tile([128, 1], BF16, tag="vt")
            nc.scalar.copy(vt[:], pt[:])
            nc.vector.tensor_copy(vgm[0:64, cc, 1 + 2 * cc:2 + 2 * cc],
                                  vt[0:64, :])
            nc.vector.tensor_copy(vgm[64:128, cc, 2 + 2 * cc:3 + 2 * cc],
                                  vt[64:128, :])
        for jh in range(2):
            ps_wg = pa_pool.tile([H + 1, 512], F32, tag="pa")
            for cc in range(8):
                nc.tensor.matmul(ps_wg[:], vgm[:, cc, :],
                                 woT[:, cc, 512 * jh:512 * jh + 512],
                                 start=(cc == 0), stop=(cc == 7))
            nc.scalar.copy(wg[:, 512 * jh:512 * jh + 512], ps_wg[:])
        nc.vector.tensor_copy(wg[0:1, :], brow["bo"][:])

        # global query scored against global key
        ps_sg0 = ps_s_pool.tile([1, H], F32, tag="ps_s")
        for jc in range(8):
            nc.tensor.matmul(ps_sg0[:], k0[:, jc:jc + 1], qgbd[:, jc, :],
                             start=(jc == 0), stop=(jc == 7))
        esg0 = sb_z_pool.tile([1, H], BF16, tag="esg0")
        nc.scalar.activation(esg0[:], ps_sg0[:], AF.Exp)
        nc.vector.tensor_scalar_mul(esg0m[:], esg0[:], gq[0:1, 0:1])

        # ================= streaming loop over 512-token groups =========
        for tm in range(4):
            hs_t = get_hs(tm)
            qT_t = q_pool.tile([128, 8, 512], BF16, tag="qT")
            kT_t = k_pool.tile([128, 8, 2, LB], BF16, tag="kT")
            v_t = v_pool.tile([128, 4, C], BF16, tag="v")

            # ---- q / k projections for this group ----
            for tgt in ("q", "k"):
                for w2 in range(2):
                    for jc2 in range(4):
                        jc = 4 * w2 + jc2
                        ps = pa_pool.tile([128, 512], F32, tag="pa")
                        for cc in range(8):
                            nc.tensor.matmul(
                                ps[:],
                                wh[tgt, w2][:, cc, 128 * jc2:128 * jc2 + 128],
                                hs_t[:, cc, :],
                                start=(cc == 0), stop=(cc == 7))
                        if tgt == "q":
                            nc.vector.tensor_scalar_add(
                                qT_t[:, jc, :], ps[:],
                                bcol["bq"][:, jc:jc + 1])
                        else:
                            nc.vector.tensor_scalar_add(
                                kT_t[:, jc, :, 0:L],
                                ps[:].rearrange("p (f c) -> p f c", c=L),
                                bcol["bk"][:, jc:jc + 1])
                    if tgt == "k":
                        for jc2 in range(4):
                            jc = 4 * w2 + jc2
                            for kc in range(2):
                                nc.vector.tensor_copy(
                                    kT_t[:, jc, kc, L:LB], k0[:, jc:jc + 1])

            # ---- v projection for this group ----
            for i in range(4):
                for jh in range(2):
                    ps = pa_pool.tile([128, 512], F32, tag="pa")
                    for cc in range(8):
                        nc.tensor.matmul(
                            ps[:], hs_t[:, cc, 128 * i:128 * i + 128],
                            wh["v", jh][:, cc, :],
                            start=(cc == 0), stop=False)
                    nc.tensor.matmul(
                        ps[:], ones1[0:1, :],
                        brow["bv"][0:1, 512 * jh:512 * jh + 512],
                        start=False, stop=True)
                    nc.scalar.copy(v_t[:, i, 512 * jh:512 * jh + 512], ps[:])

            # ---- global-query scores for this group ----
            for i in range(4):
                tn = 4 * tm + i
                ps_sg = ps_s_pool.tile([128, H], F32, tag="ps_s")
                for jc in range(8):
                    nc.tensor.matmul(
                        ps_sg[:], kT_t[:, jc, i // 2,
                                       128 * (i % 2):128 * (i % 2) + 128],
                        qgbd[:, jc, :], start=(jc == 0), stop=(jc == 7))
                nc.scalar.activation(esg[:, tn, :], ps_sg[:], AF.Exp)

            # ---- attention + fused out-projection for the two frames ----
            def attn_head_pair(fl, f, jc):
                ps_o = ps_o_pool.tile([128, 256], F32, tag="ps_o")
                for hh in range(2):
                    h = 2 * jc + hh
                    hoff = 64 * hh
                    for qt in range(2):
                        q0c = 256 * fl + 128 * qt
                        ps_s = ps_s_pool.tile([128, LB], F32, tag="ps_s")
                        nc.tensor.matmul(
                            ps_s[:],
                            qT_t[hoff:hoff + 64, jc, q0c:q0c + 128],
                            kT_t[hoff:hoff + 64, jc, fl, :],
                            start=True, stop=True)
                        pe = sb_p_pool.tile([128, LB], BF16, tag="pe")
                        z = sb_z_pool.tile([128, 1], F32, tag="z")
                        nc.scalar.activation(pe[:], ps_s[:], AF.Exp,
                                             accum_out=z[:])
                        rz = sb_z_pool.tile([128, 1], F32, tag="rz")
                        nc.vector.reciprocal(rz[:], z[:])
                        pm = sb_p_pool.tile([128, LB], BF16, tag="pm")
                        nc.vector.tensor_scalar_mul(pm[:], pe[:], rz[:])
                        nc.vector.tensor_copy(
                            Cm[:, 2 * f + qt, 1 + h:2 + h], pm[:, L:LB])
                        for kc in range(2):
                            ps_pt = ps_pt_pool.tile([128, 128], BF16,
                                                    tag="ps_pt")
                            nc.tensor.transpose(
                                ps_pt[:], pm[:, 128 * kc:128 * kc + 128],
                                ident[:])
                            pt = sb_pt_pool.tile([128, 128], BF16, tag="pt")
                            nc.vector.tensor_copy(pt[:], ps_pt[:])
                            nc.tensor.matmul(
                                ps_o[hoff:hoff + 64,
                                     128 * qt:128 * qt + 128],
                                v_t[:, 2 * fl + kc,
                                    128 * jc + hoff:128 * jc + hoff + 64],
                                pt[:],
                                start=(kc == 0), stop=(kc == 1))
                c0 = 256 * f
                nc.scalar.copy(OT[:, jc, c0:c0 + 256], ps_o[:])

            for fl in range(2):
                f = 2 * tm + fl
                for jc in range(8):
                    attn_head_pair(fl, f, jc)

                for qt in range(2):
                    tn = 2 * f + qt
                    ps_ct = ps_pt_pool.tile([H + 1, 128], BF16, tag="ps_pt")
                    nc.tensor.transpose(ps_ct[:], Cm[:, tn, :], ident[:])
                    ct = sb_ct_pool.tile([H + 1, 128], BF16, tag="ct")
                    nc.vector.tensor_copy(ct[:], ps_ct[:])
                    for jh in range(2):
                        ps_y = ps_y_pool.tile([128, 512], F32, tag="ps_y")
                        for jc in range(8):
                            nc.tensor.matmul(
                                ps_y[:], OT[:, jc, 128 * tn:128 * tn + 128],
                                woT[:, jc, 512 * jh:512 * jh + 512],
                                start=(jc == 0), stop=False)
                        nc.tensor.matmul(ps_y[:], ct[:],
                                         wg[:, 512 * jh:512 * jh + 512],
                                         start=False, stop=True)
                        yt = sb_y_pool.tile([128, 512], F32, tag="yt")
                        nc.vector.tensor_copy(yt[:], ps_y[:])
                        nc.sync.dma_start(
                            io["y"][1 + 128 * tn:1 + 128 * tn + 128,
                                    512 * jh:512 * jh + 512], yt[:])

            # ---- global-query oG partial for this group ----
            for jc in range(8):
                ps_og = ps_o_pool.tile([128, 2], F32, tag="ps_o")
                for i in range(4):
                    nc.tensor.matmul(ps_og[:],
                                     v_t[:, i, 128 * jc:128 * jc + 128],
                                     esg[:, 4 * tm + i, 2 * jc:2 * jc + 2],
                                     start=(i == 0), stop=(i == 3))
                nc.vector.tensor_tensor(
                    out=og_acc[:, 2 * jc:2 * jc + 2],
                    in0=og_acc[:, 2 * jc:2 * jc + 2], in1=ps_og[:],
                    op=mybir.AluOpType.add)

        # ================= tail: zG, oG v0 term, outputs =================
        ps_z = ps_y_pool.tile([H, 1], F32, tag="ps_y")
        for tn in range(16):
            nc.tensor.matmul(ps_z[:], esg[:, tn, :], onesc[:],
                             start=(tn == 0), stop=False)
        nc.tensor.matmul(ps_z[:], esg0m[:], onesc[0:1, :],
                         start=False, stop=True)
        nc.scalar.copy(zg[:], ps_z[:])

        for jc in range(8):
            ps_og = ps_o_pool.tile([128, 2], F32, tag="ps_o")
            nc.tensor.matmul(ps_og[:], v0[0:1, 128 * jc:128 * jc + 128],
                             esg0m[0:1, 2 * jc:2 * jc + 2],
                             start=True, stop=True)
            nc.vector.tensor_tensor(
                out=ps_og[:], in0=ps_og[:],
                in1=og_acc[:, 2 * jc:2 * jc + 2], op=mybir.AluOpType.add)
            nc.vector.tensor_copy(ogp[0:64, jc:jc + 1], ps_og[0:64, 0:1])
            nc.vector.tensor_copy(ogp[64:128, jc:jc + 1], ps_og[64:128, 1:2])
        nc.sync.dma_start(io["ogp"][:, :], ogp[:])
        nc.sync.dma_start(io["zgp"][:, :], zg[:])


_NC_CACHE = {}


def build_nc():
    if "nc" in _NC_CACHE:
        return _NC_CACHE["nc"]
    nc = bacc.Bacc("TRN2", target_bir_lowering=False, debug=False,
                   num_devices=NCORES)
    io = {
        "hsT": nc.dram_tensor("hsT", [C, TL], BF16, kind="ExternalInput").ap(),
        "wqT": nc.dram_tensor("wqT", [C, C], BF16, kind="ExternalInput").ap(),
        "wkT": nc.dram_tensor("wkT", [C, C], BF16, kind="ExternalInput").ap(),
        "wvT": nc.dram_tensor("wvT", [C, C], BF16, kind="ExternalInput").ap(),
        "woT": nc.dram_tensor("woT", [C, C], BF16, kind="ExternalInput").ap(),
        "bq": nc.dram_tensor("bq", [C], F32, kind="ExternalInput").ap(),
        "bk": nc.dram_tensor("bk", [C], F32, kind="ExternalInput").ap(),
        "bv": nc.dram_tensor("bv", [C], F32, kind="ExternalInput").ap(),
        "bo": nc.dram_tensor("bo", [C], F32, kind="ExternalInput").ap(),
        "gqm": nc.dram_tensor("gqm", [1, 1], F32, kind="ExternalInput").ap(),
        "y": nc.dram_tensor("y", [TL, C], F32, kind="ExternalOutput").ap(),
        "ogp": nc.dram_tensor("ogp", [128, 8], F32, kind="ExternalOutput").ap(),
        "zgp": nc.dram_tensor("zgp", [16, 1], F32, kind="ExternalOutput").ap(),
    }
    with tile.TileContext(nc) as tc:
        with ExitStack() as ctx:
            _emit(ctx, tc, io)
    nc.compile()
    _NC_CACHE["nc"] = nc
    return nc


def make_in_maps(hidden_states, Wq, bq, Wk, bk, Wv, bv, Wo, bo):
    """Host-side sharding: per-core input dicts."""
    hs = np.asarray(hidden_states, dtype=np.float32)
    wqT = np.ascontiguousarray((np.asarray(Wq).T * SCALE)).astype(NPBF16)
    wkT = np.ascontiguousarray(np.asarray(Wk).T).astype(NPBF16)
    wvT = np.ascontiguousarray(np.asarray(Wv).T).astype(NPBF16)
    woT = np.ascontiguousarray(np.asarray(Wo).T).astype(NPBF16)
    bq_s = (np.asarray(bq, np.float32) * SCALE).astype(np.float32)
    bk_ = np.asarray(bk, np.float32)
    bv_ = np.asarray(bv, np.float32)
    bo_ = np.asarray(bo, np.float32)

    in_maps = []
    for core in range(NCORES):
        b, th = core // 2, core % 2
        hsb = hs[b]  # [T, C]
        sl = hsb[1 + th * FT: 1 + (th + 1) * FT]  # [2048, C]
        shard = np.concatenate([hsb[0:1], sl], axis=0)  # [2049, C]
        hsT = np.ascontiguousarray(shard.T).astype(NPBF16)  # [C, 2049]
        in_maps.append({
            "hsT": hsT,
            "wqT": wqT, "wkT": wkT, "wvT": wvT, "woT": woT,
            "bq": bq_s, "bk": bk_, "bv": bv_, "bo": bo_,
            "gqm": np.array([[1.0 if th == 0 else 0.0]], np.float32),
        })
    return in_maps


def assemble(results, Wo, bo):
    """Host-side gather: concat per-core halves; combine global-query row."""
    Wo = np.asarray(Wo, np.float32)
    bo_ = np.asarray(bo, np.float32)
    out = np.empty((B, T, C), np.float32)
    for b in range(B):
        r0, r1 = results[2 * b], results[2 * b + 1]
        out[b, 1:1 + FT] = r0["y"][1:]
        out[b, 1 + FT:] = r1["y"][1:]
        og = r0["ogp"] + r1["ogp"]          # [128, 8]
        zgs = (r0["zgp"] + r1["zgp"])[:, 0]  # [16]
        vec = np.empty((C,), np.float32)
        for h in range(H):
            jc, hoff = h // 2, 64 * (h % 2)
            vec[h * DH:(h + 1) * DH] = og[hoff:hoff + 64, jc] / zgs[h]
        out[b, 0] = vec @ Wo.T + bo_
    return out


_LAST_RESULTS = {}


def kernel(hidden_states, Wq, bq, Wk, bk, Wv, bv, Wo, bo, M, N, L,
           trace=False):
    assert int(M) == 1 and int(N) == 16 and int(L) == 256
    nc = build_nc()
    in_maps = make_in_maps(hidden_states, Wq, bq, Wk, bk, Wv, bv, Wo, bo)
    res = run_bass_kernel_spmd(nc, in_maps, core_ids=list(range(NCORES)),
                               trace=trace)
    _LAST_RESULTS["res"] = res
    return assemble(res.results, Wo, bo)
